# revision 71
# baseline (speedup 1.0000x reference)
"""Trainium2 Bass kernel: GQA attention layer (RoPE + causal attention + projections).

Strategy (8 NeuronCores, tensor-parallel by head):
  - Each core owns 2 query heads + 1 kv head (NH=16, NKV=8 -> GQA pairs align
    with cores exactly). QKV projection, RoPE, and attention for those heads run
    fully locally -- zero K/V communication.
  - Attention is computed in the S^T orientation ([keys, q]) so the probability
    matrix feeds the PV matmul directly as the moving operand (no transposes).
    Softmax denominator accumulates via an all-ones stationary matmul on the PE
    (4-way DVE pre-reduction); normalization is applied after PV.
  - After attention, one AllToAll per batch reshards activations from
    head-sharded to token-sharded; each core then runs o_proj for its 512
    tokens and the host concatenates the 8 slices.
  - Precision: the q/k chains run in fp8e4 DoubleRow matmuls (2x PE rate) --
    their quantization noise only perturbs attention scores, which is
    invisible at the output for any realistic score scale. The v chain,
    PV, and o_proj stay bf16 (their noise passes straight to the output).
    Scales: hidden x64, w_q x(SCALE*64), w_k x8 -> scores arrive x2^21;
    the exp activation applies 2^-21 on its input for free.
  - Software pipelining: attention chunk instructions for group g's strips
    are interleaved into group g+1's projection instruction stream (and the
    last group's strips into o_proj p0), so the in-order PE queue never
    stalls on the scalar-engine exp between a chunk's QK and PV matmuls.
    PV/den also trail QK by one chunk inside each strip.
"""

import os
from collections import deque
from contextlib import ExitStack

import ml_dtypes
import numpy as np

import concourse.bass as bass
import concourse.tile as tile
from concourse import bacc, mybir
from concourse.bass_utils import run_bass_kernel_spmd

# Problem shapes (hardcoded per spec nn_AvaAttention_36249523978775).
B, T, HID = 2, 2048, 2048
NH, NKV, HD = 16, 8, 128
SCALE = HD ** -0.5
NC = 8
TT = B * T  # 4096 flat tokens, b-major
NEG = -2.3819763e38

F32 = mybir.dt.float32
BF = mybir.dt.bfloat16
F8 = mybir.dt.float8e4
NPBF = ml_dtypes.bfloat16
NPF8 = ml_dtypes.float8_e4m3

DR = mybir.MatmulPerfMode.DoubleRow

S_H = 64.0     # fp8 scale on hidden_states
S_WQ = 64.0    # fp8 scale on w_q (on top of SCALE)
S_WK = 8.0     # fp8 scale on w_k
EXP_SCALE = 1.0 / (S_H * S_WQ * S_H * S_WK)  # 2^-21

TN = 512           # token chunk for projection moving operand
NG = TT // TN      # 8 projection token groups
NHC = HID // 128   # 16 contraction chunks (bf16 v stream)
NC2 = NHC // 2     # 8 paired contraction chunks (fp8 DoubleRow q/k streams)
NQC = T // 256     # 8 query strips of 256 per batch
NKC = T // 128     # 16 key chunks of 128 per batch

_CACHE = {}
last_results = None  # test harness reads exec_time_ns from here


def _build(mode: str):
    """Build the SPMD graph. mode in {"causal", "none", "generic"}."""
    nc = bacc.Bacc("TRN2", target_bir_lowering=False, debug=False, num_devices=NC)

    hT8_e = nc.declare_dram_parameter("hT8", [NG, 128, NC2, 2, TN], F8, isOutput=False)
    hTB_e = nc.declare_dram_parameter("hTB", [NG, 128, NHC, TN], BF, isOutput=False)
    w8_e = nc.declare_dram_parameter("w8", [128, 3, NC2, 2, 128], F8, isOutput=False)
    wv_e = nc.declare_dram_parameter("wv", [128, NHC, 128], BF, isOutput=False)
    woT_e = nc.declare_dram_parameter("woT", [NH * HD, HID], BF, isOutput=False)
    ropeC_e = nc.declare_dram_parameter("ropeC", [128, T], BF, isOutput=False)
    ropeS_e = nc.declare_dram_parameter("ropeS", [128, T], BF, isOutput=False)
    ones_e = nc.declare_dram_parameter("ones", [128, 128], BF, isOutput=False)
    ident_e = nc.declare_dram_parameter("ident", [128, 128], BF, isOutput=False)
    pat_e = None
    maskT_e = None
    if mode == "causal":
        pat_e = nc.declare_dram_parameter("pat", [2, 128, 2, 256], BF, isOutput=False)
    elif mode == "generic":
        maskT_e = nc.declare_dram_parameter("maskT", [T, T], F32, isOutput=False)
    out_e = nc.declare_dram_parameter("out", [512, HID], BF, isOutput=True)

    with tile.TileContext(nc) as tc:
        with tc.tile_pool(name="consts", bufs=1) as consts, \
             tc.tile_pool(name="dram", bufs=1, space="DRAM") as dram:

            ones_t = consts.tile([128, 128], BF)
            ident_t = consts.tile([128, 128], BF)
            pat_t = None
            if mode == "causal":
                # bf16 is fine for the mask: it represents -2.38e38 and the
                # add upconverts into the f32 scores
                pat_t = consts.tile([128, 2, 2, 256], BF)

            # Token-split resharding: each collective covers a contiguous
            # token range of one batch; within it core j owns the contiguous
            # block [j*W, +W). The last two collectives are HALF-sized (2
            # strips, W=64) so the final collective + its o_proj sliver are
            # all that trail the last attention strip. Contiguous token runs
            # keep the SBUF->DRAM staging DMAs at >=128B packets.
            # (b, first strip, #strips, out_e row base)
            COLLS = [(0, 0, 4, 0), (0, 4, 4, 128), (1, 0, 4, 256),
                     (1, 4, 4, 384)]
            a2a_in = [dram.tile([NC, 256, 32 * s], BF, name=f"a2a_in{i}")
                      for i, (_, _, s, _) in enumerate(COLLS)]
            a2a_out = [dram.tile([NC, 256, 32 * s], BF, name=f"a2a_out{i}")
                       for i, (_, _, s, _) in enumerate(COLLS)]

            def coll_of(b, qc):
                return 2 * b + qc // 4

            # o_proj weights: tiles reserved early (pool-nesting order), DMA
            # emitted mid-Phase A so it doesn't crowd the startup stream.
            es_wo = ExitStack()
            wop = es_wo.enter_context(tc.tile_pool(name="wop", bufs=1))
            wo_res = [wop.tile([128, NH, 1024], BF, name=f"wo{half}")
                      for half in range(2)]

            es = ExitStack()
            big = es.enter_context(tc.tile_pool(name="big", bufs=1))
            # Persistent activations (my heads, all tokens).
            # q8/k8 hold the d dim split in two 64-row k-tiles for DoubleRow
            # matmuls; q8 packs both heads per 256-token strip so a strip's
            # moving operand is one contiguous [64, 2, 512] slice. 512 slack
            # columns let the strided RoPE write slice [base, base+1024).
            q8_sb = big.tile([64, 2, B * NQC * 512 + 512], F8)
            k8_sb = big.tile([64, 2, TT], F8)              # [d%64, d//64, tok]
            v_sb = big.tile([128, TT // 128, 128], BF)     # V natural, [tok-chunk, d]

            # attention-side pools stay open through the o_proj-p0 overlap
            es_att = ExitStack()
            psS = es_att.enter_context(tc.tile_pool(name="psS", bufs=3, space="PSUM"))
            psPV = es_att.enter_context(tc.tile_pool(name="psPV", bufs=1, space="PSUM"))
            psDen = es_att.enter_context(tc.tile_pool(name="psDen", bufs=1, space="PSUM"))
            pt_pool = es_att.enter_context(tc.tile_pool(name="pt", bufs=5))
            attev = es_att.enter_context(tc.tile_pool(name="attev", bufs=2))
            mt_pool = (es_att.enter_context(tc.tile_pool(name="mt", bufs=3))
                       if mode == "generic" else None)

            def strip_chunks(b, qc):
                """Generator: one yield per pipeline stage. PV/den for chunk
                ci are emitted at stage ci+1 so they sit behind the next QK
                in the PE queue, past the exp latency."""
                cmax = 2 * qc + 2 if mode == "causal" else NKC
                mv = q8_sb[:, :, (b * NQC + qc) * 512:(b * NQC + qc) * 512 + 512]
                pv = psPV.tile([128, 512], F32, name="pv", tag="pv")
                den = psDen.tile([128, 512], F32, name="den", tag="den")
                pend = []          # un-reduced pt tiles (4-way DVE reduction)
                nden = (cmax + 3) // 4
                dci = 0

                # causal mode: the odd diagonal chunk (ci == 2qc+1, always the
                # last) fully masks query offsets [0,128) of both heads, so
                # its QK/exp/PV/den run at HALF width, accumulating into the
                # query-half-2 slices of pv/den.
                def half_q(ap3):
                    return ap3.rearrange("p (h q) -> p h q", q=256)[:, :, 128:256]

                def pv_den(ci, pt):
                    nonlocal dci
                    if mode == "causal" and ci == cmax - 1:
                        nc.tensor.matmul(half_q(pv[:]), v_sb[:, NKC * b + ci, :],
                                         pt[:], start=False, stop=True)
                        while len(pend) > 1:
                            ps2 = pt_pool.tile([128, 512], BF, name="ps2", tag="ps2")
                            nc.vector.tensor_add(ps2[:], pend[-2][:], pend[-1][:])
                            pend[-2:] = [ps2]
                        if pend:
                            nc.tensor.matmul(den[:], ones_t[:], pend[0][:],
                                             start=(dci == 0), stop=False)
                            pend.clear()
                            dci += 1
                        nc.tensor.matmul(half_q(den[:]), ones_t[:], pt[:],
                                         start=False, stop=True)
                        return
                    nc.tensor.matmul(pv[:], v_sb[:, NKC * b + ci, :], pt[:],
                                     start=(ci == 0), stop=(ci == cmax - 1))
                    if len(pend) == 2:
                        ps2 = pt_pool.tile([128, 512], BF, name="ps2", tag="ps2")
                        nc.vector.tensor_add(ps2[:], pend[0][:], pend[1][:])
                        pend[:] = [ps2]
                    pend.append(pt)
                    if ci % 4 == 3 or ci == cmax - 1:
                        if len(pend) == 2:
                            ps2 = pt_pool.tile([128, 512], BF, name="ps2", tag="ps2")
                            nc.vector.tensor_add(ps2[:], pend[0][:], pend[1][:])
                            pend[:] = [ps2]
                        nc.tensor.matmul(den[:], ones_t[:], pend[0][:],
                                         start=(dci == 0),
                                         stop=(dci == nden - 1 and mode != "causal"))
                        pend.clear()
                        dci += 1

                prev = None
                for ci in range(cmax):
                    if mode == "causal" and ci == 2 * qc + 1:
                        st = psS.tile([128, 256], F32, name="st", tag="st")
                        base = (b * NQC + qc) * 512
                        mvh = q8_sb[:, :, base:base + 512] \
                            .rearrange("p k (h q) -> p k h q", q=256)[:, :, :, 128:256]
                        nc.tensor.matmul(
                            st[:], k8_sb[:, :, b * T + 128 * ci: b * T + 128 * ci + 128],
                            mvh, start=True, stop=True, perf_mode=DR)
                        st3 = st[:].rearrange("p (h q) -> p h q", q=128)
                        nc.vector.tensor_add(st3, st3, pat_t[:, 1, :, 128:256])
                        pt = pt_pool.tile([128, 256], BF, name="pt", tag="pt")
                    else:
                        st = psS.tile([128, 512], F32, name="st", tag="st")
                        nc.tensor.matmul(st[:], k8_sb[:, :, b * T + 128 * ci: b * T + 128 * ci + 128],
                                         mv, start=True, stop=True, perf_mode=DR)
                        if mode == "causal" and ci == 2 * qc:
                            nc.vector.tensor_add(
                                st[:], st[:],
                                pat_t[:, 0, :, :].rearrange("p h t -> p (h t)"))
                        elif mode == "generic":
                            mt = mt_pool.tile([128, 256], F32, name="mt", tag="mt")
                            nc.sync.dma_start(
                                mt[:], maskT_e[128 * ci:128 * ci + 128,
                                               256 * qc:256 * qc + 256])
                            nc.vector.tensor_add(st[:, 0:256], st[:, 0:256], mt[:])
                            nc.vector.tensor_add(st[:, 256:512], st[:, 256:512], mt[:])
                        pt = pt_pool.tile([128, 512], BF, name="pt", tag="pt")
                    nc.scalar.activation(pt[:], st[:],
                                         mybir.ActivationFunctionType.Exp,
                                         scale=EXP_SCALE)
                    if prev is not None:
                        pv_den(*prev)
                    prev = (ci, pt)
                    yield
                pv_den(*prev)
                # den rows are all identical (ones stationary) == softmax denom
                den_rb = attev.tile([128, 512], F32, name="den_rb", tag="den_rb")
                nc.vector.reciprocal_approx_fast(den_rb[:], den[:])
                ao = attev.tile([128, 512], BF, name="ao", tag="ao")
                nc.vector.tensor_mul(ao[:], pv[:], den_rb[:])
                # strip (b,qc) covers tokens [qc*256, +256) of batch b; inside
                # its collective ci core j owns contiguous block [j*W, +W).
                # Staged on the scalar DGE ring so the collective triggers
                # never wait behind the big sync-ring loads or the
                # collective-gated att_g reads.
                ci = coll_of(b, qc)
                _, q0, S, _ = COLLS[ci]
                W = 32 * S
                ns = 256 // W  # slots this strip covers
                for h in range(2):  # ONE 3-dim DMA per head: each dma_start
                    # costs ~0.5us of scalar-engine issue time, so coalesce
                    nc.scalar.dma_start(
                        a2a_in[ci][(qc - q0) * ns:(qc - q0 + 1) * ns,
                                   128 * h:128 * h + 128, :]
                        .rearrange("s p q -> p s q"),
                        ao[:, 256 * h:256 * h + 256]
                        .rearrange("p (s q) -> p s q", q=W))
                done_strips[ci] += 1

            pending = deque()  # (batch, generator) of in-flight strips
            done_strips = {ci: 0 for ci in range(len(COLLS))}
            a2a_done = {ci: False for ci in range(len(COLLS))}

            def fire_ready_a2a():
                for ci, n in done_strips.items():
                    if n == COLLS[ci][2] and not a2a_done[ci]:
                        a2a_done[ci] = True
                        nc.gpsimd.collective_compute(
                            "AllToAll", mybir.AluOpType.bypass,
                            replica_groups=[list(range(NC))],
                            ins=[a2a_in[ci][:].opt()],
                            outs=[a2a_out[ci][:].opt()])

            budget = [10 ** 9]  # per-group stage cap; levels DVE/scalar load

            def drain(n):
                done = 0
                while pending and done < n and budget[0] > 0:
                    try:
                        next(pending[0][1])
                        done += 1
                        budget[0] -= 1
                    except StopIteration:
                        pending.popleft()
                        # fire each collective at the earliest emission point
                        # -- minimizes how much other queue traffic its
                        # trigger semaphores transitively wait on
                        fire_ready_a2a()

            def drain_all():
                budget[0] = 10 ** 9
                while pending:
                    drain(1)

            # -------- Phase A+B interleaved: projection feeds attention ------
            es_proj = ExitStack()
            wrope = es_proj.enter_context(tc.tile_pool(name="wrope", bufs=1))
            ht8_pool = es_proj.enter_context(tc.tile_pool(name="ht8", bufs=2))
            htB_pool = es_proj.enter_context(tc.tile_pool(name="htB", bufs=1))
            psA = es_proj.enter_context(tc.tile_pool(name="psA", bufs=2, space="PSUM"))
            psTr = es_proj.enter_context(tc.tile_pool(name="psTr", bufs=1, space="PSUM"))
            rtmp = es_proj.enter_context(tc.tile_pool(name="ropetmp", bufs=2))
            vtmp = es_proj.enter_context(tc.tile_pool(name="vtmp", bufs=2))

            ropeC_t = wrope.tile([128, T], BF)
            ropeS_t = wrope.tile([128, T], BF)
            # one tile per stream: a multi-DMA load into a single tile
            # collapses to a whole-tile dependency, which would gate the
            # first matmul on ALL of w8 instead of just stream 0
            w8s = [wrope.tile([128, NC2, 2, 128], F8, name=f"w8s{s}")
                   for s in range(3)]
            wv_t = wrope.tile([128, NHC, 128], BF)

            h8_next = None
            for g in range(NG):
                t0 = g * TN
                b_g = g // (T // TN)
                # cap stage drains so a heavy strip pair (up to 30 chunks)
                # spills into the next, lighter group instead of saturating
                # this group's DVE/scalar
                budget[0] = 24
                if g == 0:
                    # first-needed first on the sync ring: stream-0 weights,
                    # group-0 fp8 activations chunk-by-chunk (separate tiles
                    # so the first matmul starts on partial data), remaining
                    # weights. One-time loads (rope tables, v weights,
                    # consts) ride the idle scalar ring so they don't push
                    # group-1's loads back.
                    nc.sync.dma_start(w8s[0][:], w8_e[:, 0])
                    nc.scalar.dma_start(ropeC_t[:], ropeC_e[:])
                    nc.scalar.dma_start(ropeS_t[:], ropeS_e[:])
                    # 4 double-chunks (fewer dma_start issues, each ~0.5us of
                    # engine time); streams 1/2's weights right after the
                    # first chunk so no stream ever waits on weights
                    h8_parts = [ht8_pool.tile([128, 2, 2, TN], F8,
                                              name=f"h8p{p}", bufs=1)
                                for p in range(4)]
                    nc.sync.dma_start(h8_parts[0][:], hT8_e[0][:, 0:2])
                    nc.sync.dma_start(w8s[1][:], w8_e[:, 1])
                    nc.sync.dma_start(w8s[2][:], w8_e[:, 2])
                    for p in range(1, 4):
                        nc.sync.dma_start(h8_parts[p][:],
                                          hT8_e[0][:, 2 * p:2 * p + 2])
                    h8_at = lambda c2: h8_parts[c2 // 2][:, c2 % 2]
                    nc.scalar.dma_start(wv_t[:], wv_e[:])
                    nc.scalar.dma_start(ident_t[:], ident_e[:])
                    nc.scalar.dma_start(ones_t[:], ones_e[:])
                    if mode == "causal":
                        nc.scalar.dma_start(
                            pat_t[:], pat_e[:].rearrange("s p h t -> p s h t"))
                else:
                    h8_g = h8_next
                    h8_at = lambda c2: h8_g[:, c2]
                # prefetch the NEXT group's fp8 activations ahead of this
                # group's v-stream load: the q/k streams of g+1 never wait.
                # Exception g0: its own ht gates the v stream ~8us earlier
                # than g1 needs h8, so ht goes first there.
                def prefetch_h8():
                    nonlocal h8_next
                    h8_next = ht8_pool.tile([128, NC2, 2, TN], F8, name="h8",
                                            tag="h8")
                    nc.sync.dma_start(h8_next[:], hT8_e[g + 1])
                if 0 < g < NG - 1:
                    prefetch_h8()
                if g == 0:
                    # two half-tiles so the v stream starts on the first MB
                    # (funded by htB bufs=1: the h8 prefetch-one-ahead means
                    # ht no longer needs double buffering)
                    ht_parts = [htB_pool.tile([128, NHC // 2, TN], BF,
                                              name=f"htp{k}", bufs=1)
                                for k in range(2)]
                    for k in range(2):
                        nc.sync.dma_start(ht_parts[k][:],
                                          hTB_e[0][:, 8 * k:8 * k + 8, :])
                    ht_at = lambda hc: ht_parts[hc // 8][:, hc % 8, :]
                    prefetch_h8()
                else:
                    ht_g = htB_pool.tile([128, NHC, TN], BF, name="ht",
                                         tag="ht")
                    nc.sync.dma_start(ht_g[:], hTB_e[g])
                    ht_at = lambda hc: ht_g[:, hc, :]
                if g in (3, 5):
                    half = (g - 3) // 2
                    nc.sync.dma_start(
                        wo_res[half][:],
                        woT_e[:, half * 1024:(half + 1) * 1024]
                        .rearrange("(h p) n -> p h n", p=128))
                ctab = g % (T // TN) * TN  # rope table column offset

                def rope_out(s, ps):
                    # RoPE: out = ps*C + rot(ps)*S  (S carries the sign),
                    # written as fp8 d-halves for the DoubleRow QK matmul.
                    # The chain runs in bf16 (2x DVE rate) behind one scalar
                    # copy; q/k noise only lands on attention scores.
                    pb = rtmp.tile([128, TN], BF, name="pb", tag="pb")
                    nc.scalar.copy(pb[:], ps[:])
                    csl = ropeC_t[:, ctab:ctab + TN]
                    ssl = ropeS_t[:, ctab:ctab + TN]
                    t1 = rtmp.tile([128, TN], BF, name="t1", tag="t1")
                    t2 = rtmp.tile([128, TN], BF, name="t2", tag="t2")
                    # ropeS halves are swapped host-side so each mul reads
                    # both SBUF inputs from the same base partition
                    nc.vector.tensor_mul(t1[:], pb[:], csl)
                    nc.vector.tensor_mul(t2[0:64, :], pb[64:128, :], ssl[64:128, :])
                    nc.vector.tensor_mul(t2[64:128, :], pb[0:64, :], ssl[0:64, :])
                    if s < 2:
                        # strip-packed layout: head s of strip qc occupies
                        # tokens [qc*512 + s*256, +256) -- this group's two
                        # strips land at base+{0,512} (stride-512 write)
                        base = (b_g * NQC + 2 * (g % 4)) * 512 + s * 256
                        for k in range(2):
                            dst = q8_sb[:, k, base:base + 1024] \
                                .rearrange("p (q t) -> p q t", t=512)[:, :, 0:256]
                            nc.vector.tensor_add(
                                dst,
                                t1[64 * k:64 * k + 64, :].rearrange("p (q t) -> p q t", t=256),
                                t2[64 * k:64 * k + 64, :].rearrange("p (q t) -> p q t", t=256))
                    else:
                        for k in range(2):
                            nc.vector.tensor_add(
                                k8_sb[:, k, t0:t0 + TN],
                                t1[64 * k:64 * k + 64, :],
                                t2[64 * k:64 * k + 64, :])

                if g == 0:
                    # cold start: interleave the three streams per chunk so
                    # the PE consumes activation chunks at DMA delivery rate
                    # (a stream-at-a-time order races ahead and stalls). The
                    # third accumulator borrows the psPV bank, idle until the
                    # first strip drains in group 1.
                    pss = [psA.tile([128, TN], F32, name="psA", tag="psA"),
                           psA.tile([128, TN], F32, name="psA", tag="psA"),
                           psPV.tile([128, TN], F32, name="pv", tag="pv")]
                    for c2 in range(NC2):
                        for s in range(3):
                            nc.tensor.matmul(pss[s][:], w8s[s][:, c2],
                                             h8_at(c2), start=(c2 == 0),
                                             stop=(c2 == NC2 - 1),
                                             perf_mode=DR)
                    for s in range(3):
                        rope_out(s, pss[s])
                else:
                    for s in range(3):  # q1, q2, k in fp8 DoubleRow
                        ps = psA.tile([128, TN], F32, name="psA", tag="psA")
                        for c2 in range(NC2):
                            nc.tensor.matmul(ps[:], w8s[s][:, c2], h8_at(c2),
                                             start=(c2 == 0),
                                             stop=(c2 == NC2 - 1),
                                             perf_mode=DR)
                            drain(1)
                        rope_out(s, ps)
                        drain(1)
                # v stream in bf16 (its noise lands on the output directly).
                # g0 borrows the psDen bank (first used by strip (0,0)'s den
                # flush mid-g1) so both psA slots recycle to g1's streams off
                # the fast pb copies instead of waiting for g0's vt copy.
                if g == 0:
                    ps = psDen.tile([128, TN], F32, name="den", tag="den")
                else:
                    ps = psA.tile([128, TN], F32, name="psA", tag="psA")
                for hc in range(NHC):
                    nc.tensor.matmul(ps[:], wv_t[:, hc], ht_at(hc),
                                     start=(hc == 0), stop=(hc == NHC - 1))
                    drain(1)
                # V^T -> transpose to V natural via PE
                vt = vtmp.tile([128, TN], BF, name="vt", tag="vt")
                nc.scalar.copy(vt[:], ps[:])
                for j in range(TN // 128):
                    trp = psTr.tile([128, 128], BF, name="trp", tag="trp")
                    nc.tensor.transpose(trp[:], vt[:, j * 128:(j + 1) * 128], ident_t[:])
                    nc.vector.tensor_copy(v_sb[:, g * (TN // 128) + j, :], trp[:])
                    drain(1)
                # enqueue strips unlocked by this group (drained next group)
                if mode == "causal":
                    strips = [(b_g, 2 * (g % 4)), (b_g, 2 * (g % 4) + 1)]
                else:
                    strips = ([(g // 4, qc) for qc in range(NQC)]
                              if g in (3, 7) else [])
                for b, qc in strips:
                    pending.append((b, strip_chunks(b, qc)))
                fire_ready_a2a()

            es_proj.close()  # free projection SBUF + the psA/psTr PSUM banks

            # ---------------- Phase C: o_proj --------------------------------
            # One att_g tile per collective so each o_proj slice depends ONLY
            # on its own collective; passes are emitted in arrival order.
            with tc.tile_pool(name="attg", bufs=1) as attg_pool, \
                 tc.tile_pool(name="fin", bufs=3, space="PSUM") as fin_pool, \
                 tc.tile_pool(name="fo", bufs=3) as fo_pool:

                att_g = [attg_pool.tile([128, NH, 32 * s], BF, name=f"attg{i}")
                         for i, (_, _, s, _) in enumerate(COLLS)]

                def attg_dma(ci):
                    # alternate hw queues so the 8 transfers drain in
                    # parallel right after the collective lands
                    for j in range(NC):
                        eng = nc.sync if j % 2 == 0 else nc.scalar
                        eng.dma_start(
                            att_g[ci][:, 2 * j:2 * j + 2, :],
                            a2a_out[ci][j].rearrange("(h p) t -> p h t", p=128))

                def oproj_sub(ci, half, n2, split_out=False):
                    W = 32 * COLLS[ci][2]
                    row = COLLS[ci][3]
                    col = half * 1024 + n2 * 512
                    fin = fin_pool.tile([128, 512], F32, name="fin", tag="fin")
                    for h in range(NH):
                        nc.tensor.matmul(
                            fin[0:W, :],
                            att_g[ci][:, h, :],
                            wo_res[half][:, h, n2 * 512:(n2 + 1) * 512],
                            start=(h == 0), stop=(h == NH - 1))
                    fo = fo_pool.tile([128, 512], BF, name="fo", tag="fo")
                    if split_out:
                        # final sub-block: split the flush across both DMA
                        # rings; copies on the (idle-by-now) DVE
                        nc.vector.tensor_copy(fo[0:W, 0:256], fin[0:W, 0:256])
                        nc.vector.tensor_copy(fo[0:W, 256:512], fin[0:W, 256:512])
                        nc.sync.dma_start(out_e[row:row + W, col:col + 256],
                                          fo[0:W, 0:256])
                        nc.scalar.dma_start(out_e[row:row + W, col + 256:col + 512],
                                            fo[0:W, 256:512])
                        return
                    # the last slice runs after the strips are done: the DVE
                    # is idle there, while the scalar queue still drains attg
                    # issues -- keep its copies off the scalar engine
                    if (half + n2) % 2 == 0 or ci == len(COLLS) - 1:
                        nc.vector.tensor_copy(fo[0:W, :], fin[0:W, :])
                    else:
                        nc.scalar.copy(fo[0:W, :], fin[0:W, :])
                    nc.sync.dma_start(
                        out_e[row:row + W, col:col + 512], fo[0:W, :])

                # drain the last strips WITHOUT consuming o_proj work; each
                # o_proj slice then waits only on its own collective. One
                # sub-block of each of the first three slices is DEFERRED to
                # bridge the final collective's latency with PE work that is
                # already unlocked.
                drain_all()
                defer = [(0, 1, 1), (1, 1, 1), (2, 1, 1)]
                for ci in range(len(COLLS) - 1):
                    attg_dma(ci)
                    for half in range(2):
                        for n2 in range(2):
                            if (ci, half, n2) not in defer:
                                oproj_sub(ci, half, n2)
                for ci, half, n2 in defer:
                    oproj_sub(ci, half, n2)
                ci = len(COLLS) - 1
                attg_dma(ci)
                for half in range(2):
                    for n2 in range(2):
                        oproj_sub(ci, half, n2,
                                  split_out=(half == 1 and n2 == 1))
            es_att.close()
            es.close()
            es_wo.close()

    nc.compile()
    return nc


def _host_prep(hidden_states, freqs_cos, freqs_sin, mask, w_qkv, w_o, kv_write_indices):
    idx = np.asarray(kv_write_indices).astype(np.int64)
    if not np.array_equal(idx, np.arange(T, dtype=np.int64)):
        raise NotImplementedError("kernel specialized for kv_write_indices == arange(T)")

    hs = np.asarray(hidden_states, dtype=np.float32).reshape(TT, HID)
    hsT = hs.T  # [HID, TT]
    # bf16 copy (v stream): [HID, TT] -> [NG, 128, NHC, TN] (one DMA per group)
    hTB = np.ascontiguousarray(
        hsT.reshape(NHC, 128, NG, TN).transpose(2, 1, 0, 3)).astype(NPBF)
    # fp8 copy (q/k streams): chunk pairs for DoubleRow, one DMA per group
    # [NG, 128, NC2, 2, TN]; element (g,p,c2,k,t) = hsT[c2*256+k*128+p, g*TN+t]
    hT8 = np.ascontiguousarray(
        (hsT * S_H).reshape(NC2, 2, 128, NG, TN).transpose(3, 2, 0, 1, 4)).astype(NPF8)

    m2 = np.asarray(mask, dtype=np.float32).reshape(T, T)
    tril = np.tril(np.ones((T, T), dtype=bool))
    if not m2.any():
        mode = "none"
    elif (m2[tril] == 0).all() and (m2[~tril] <= -1e30).all():
        mode = "causal"
    else:
        mode = "generic"

    wq = np.asarray(w_qkv, dtype=np.float32)
    woT = np.ascontiguousarray(np.asarray(w_o, dtype=np.float32).T).astype(NPBF)

    def tile_w8(wrows):
        # [128 out, HID] -> [NC2, 2, 128 hid, 128 out] DoubleRow stationary
        return np.ascontiguousarray(wrows.T).reshape(NC2, 2, 128, 128)

    w8s, wvs = [], []
    for c in range(NC):
        q1 = wq[(2 * c) * HD:(2 * c + 1) * HD] * (SCALE * S_WQ)
        q2 = wq[(2 * c + 1) * HD:(2 * c + 2) * HD] * (SCALE * S_WQ)
        k = wq[NH * HD + c * HD: NH * HD + (c + 1) * HD] * S_WK
        v = wq[(NH + NKV) * HD + c * HD: (NH + NKV) * HD + (c + 1) * HD]
        # [3, NC2, 2, 128 hid, 128 out] -> [128 hid, 3, NC2, 2, 128 out]
        w8s.append(np.ascontiguousarray(
            np.stack([tile_w8(q1), tile_w8(q2), tile_w8(k)])
            .transpose(3, 0, 1, 2, 4)).astype(NPF8))
        # v: [NHC, 128 hid, 128 out] -> [128 hid, NHC, 128 out]
        wvs.append(np.ascontiguousarray(
            v.T.reshape(NHC, 128, 128).transpose(1, 0, 2)).astype(NPBF))

    cosT = np.asarray(freqs_cos, dtype=np.float32).T  # [64, T]
    sinT = np.asarray(freqs_sin, dtype=np.float32).T
    ropeC = np.ascontiguousarray(np.concatenate([cosT, cosT], axis=0)).astype(NPBF)
    # halves swapped: rows 0:64 feed t2[64:128] (+sin), rows 64:128 feed
    # t2[0:64] (-sin) -- keeps both DVE inputs on one base partition
    ropeS = np.ascontiguousarray(np.concatenate([sinT, -sinT], axis=0)).astype(NPBF)

    consts = {
        "ropeC": ropeC,
        "ropeS": ropeS,
        "ones": np.ones((128, 128), NPBF),
        "ident": np.eye(128, dtype=np.float32).astype(NPBF),
    }
    if mode == "causal":
        kr = np.arange(256)[:, None]
        qr = np.arange(256)[None, :]
        pat = np.where(kr <= qr, np.float32(0.0), np.float32(NEG)).astype(np.float32)
        pat = pat.reshape(2, 128, 1, 256).repeat(2, axis=2)  # dup over heads
        consts["pat"] = np.ascontiguousarray(pat).astype(NPBF)
    elif mode == "generic":
        # scores sit in PSUM pre-multiplied by 1/EXP_SCALE; match the mask
        consts["maskT"] = np.ascontiguousarray(
            np.clip(m2.T * (1.0 / EXP_SCALE), -3.0e38, 3.0e38))

    in_maps = []
    for c in range(NC):
        m = {"hT8": hT8, "hTB": hTB, "w8": w8s[c], "wv": wvs[c], "woT": woT}
        m.update(consts)
        in_maps.append(m)
    return mode, in_maps


def kernel(hidden_states, freqs_cos, freqs_sin, k_cache, v_cache, mask, w_qkv,
           w_o, kv_write_indices):
    # k_cache/v_cache are fully overwritten (kv_write_indices == arange covers
    # every slot), so their incoming contents are irrelevant.
    global last_results
    mode, in_maps = _host_prep(hidden_states, freqs_cos, freqs_sin, mask,
                               w_qkv, w_o, kv_write_indices)
    if mode not in _CACHE:
        _CACHE[mode] = _build(mode)
    nc = _CACHE[mode]

    trace = bool(os.environ.get("BASS_KERNEL_TRACE"))
    res = run_bass_kernel_spmd(nc, in_maps, core_ids=list(range(NC)), trace=trace)
    last_results = res

    # token-split resharding: out rows per collective, core c owning the
    # contiguous block [base_tok + c*W, +W) of that collective's batch
    # (b, base_tok within batch, W, out row base)
    colls = [(0, 0, 128, 0), (0, 1024, 128, 128), (1, 0, 128, 256),
             (1, 1024, 128, 384)]
    final = np.empty((B, T, HID), dtype=np.float32)
    for c in range(NC):
        o = np.asarray(res.results[c]["out"], dtype=np.float32)
        for b, bt, w, row in colls:
            final[b, bt + c * w: bt + (c + 1) * w] = o[row:row + w]
    return final



# revision 72
# speedup vs baseline: 1.0290x; 1.0290x over previous
"""Trainium2 Bass kernel: GQA attention layer (RoPE + causal attention + projections).

Strategy (8 NeuronCores, tensor-parallel by head):
  - Each core owns 2 query heads + 1 kv head (NH=16, NKV=8 -> GQA pairs align
    with cores exactly). QKV projection, RoPE, and attention for those heads run
    fully locally -- zero K/V communication.
  - Attention is computed in the S^T orientation ([keys, q]) so the probability
    matrix feeds the PV matmul directly as the moving operand (no transposes).
    Softmax denominator accumulates via an all-ones stationary matmul on the PE
    (4-way DVE pre-reduction); normalization is applied after PV.
  - After attention, one AllToAll per batch reshards activations from
    head-sharded to token-sharded; each core then runs o_proj for its 512
    tokens and the host concatenates the 8 slices.
  - Precision: the q/k chains run in fp8e4 DoubleRow matmuls (2x PE rate) --
    their quantization noise only perturbs attention scores, which is
    invisible at the output for any realistic score scale. The v chain,
    PV, and o_proj stay bf16 (their noise passes straight to the output).
    Scales: hidden x64, w_q x(SCALE*64), w_k x8 -> scores arrive x2^21;
    the exp activation applies 2^-21 on its input for free.
  - Software pipelining: attention chunk instructions for group g's strips
    are interleaved into group g+1's projection instruction stream (and the
    last group's strips into o_proj p0), so the in-order PE queue never
    stalls on the scalar-engine exp between a chunk's QK and PV matmuls.
    PV/den also trail QK by one chunk inside each strip.
"""

import os
from collections import deque
from contextlib import ExitStack

import ml_dtypes
import numpy as np

import concourse.bass as bass
import concourse.tile as tile
from concourse import bacc, mybir
from concourse.bass_utils import run_bass_kernel_spmd

# Problem shapes (hardcoded per spec nn_AvaAttention_36249523978775).
B, T, HID = 2, 2048, 2048
NH, NKV, HD = 16, 8, 128
SCALE = HD ** -0.5
NC = 8
TT = B * T  # 4096 flat tokens, b-major
NEG = -2.3819763e38

F32 = mybir.dt.float32
BF = mybir.dt.bfloat16
F8 = mybir.dt.float8e4
NPBF = ml_dtypes.bfloat16
NPF8 = ml_dtypes.float8_e4m3

DR = mybir.MatmulPerfMode.DoubleRow

S_H = 64.0     # fp8 scale on hidden_states
S_WQ = 64.0    # fp8 scale on w_q (on top of SCALE)
S_WK = 8.0     # fp8 scale on w_k
EXP_SCALE = 1.0 / (S_H * S_WQ * S_H * S_WK)  # 2^-21

TN = 512           # token chunk for projection moving operand
NG = TT // TN      # 8 projection token groups
NHC = HID // 128   # 16 contraction chunks (bf16 v stream)
NC2 = NHC // 2     # 8 paired contraction chunks (fp8 DoubleRow q/k streams)
NQC = T // 256     # 8 query strips of 256 per batch
NKC = T // 128     # 16 key chunks of 128 per batch

_CACHE = {}
last_results = None  # test harness reads exec_time_ns from here


def _build(mode: str):
    """Build the SPMD graph. mode in {"causal", "none", "generic"}."""
    nc = bacc.Bacc("TRN2", target_bir_lowering=False, debug=False, num_devices=NC)

    hT8_e = nc.declare_dram_parameter("hT8", [NG, 128, NC2, 2, TN], F8, isOutput=False)
    hTB_e = nc.declare_dram_parameter("hTB", [NG, 128, NHC, TN], BF, isOutput=False)
    w8_e = nc.declare_dram_parameter("w8", [128, 3, NC2, 2, 128], F8, isOutput=False)
    wv_e = nc.declare_dram_parameter("wv", [128, NHC, 128], BF, isOutput=False)
    woT_e = nc.declare_dram_parameter("woT", [NH * HD, HID], BF, isOutput=False)
    ropeC_e = nc.declare_dram_parameter("ropeC", [128, T], BF, isOutput=False)
    ropeS_e = nc.declare_dram_parameter("ropeS", [128, T], BF, isOutput=False)
    ones_e = nc.declare_dram_parameter("ones", [128, 128], BF, isOutput=False)
    ident_e = nc.declare_dram_parameter("ident", [128, 128], BF, isOutput=False)
    pat_e = None
    maskT_e = None
    if mode == "causal":
        pat_e = nc.declare_dram_parameter("pat", [2, 128, 2, 256], BF, isOutput=False)
    elif mode == "generic":
        maskT_e = nc.declare_dram_parameter("maskT", [T, T], F32, isOutput=False)
    out_e = nc.declare_dram_parameter("out", [512, HID], BF, isOutput=True)

    with tile.TileContext(nc) as tc:
        with tc.tile_pool(name="consts", bufs=1) as consts, \
             tc.tile_pool(name="dram", bufs=1, space="DRAM") as dram:

            ones_t = consts.tile([128, 128], BF)
            ident_t = consts.tile([128, 128], BF)
            pat_t = None
            if mode == "causal":
                # bf16 is fine for the mask: it represents -2.38e38 and the
                # add upconverts into the f32 scores
                pat_t = consts.tile([128, 2, 2, 256], BF)

            # Token-split resharding: each collective covers a contiguous
            # token range of one batch; within it core j owns the contiguous
            # block [j*W, +W). The last two collectives are HALF-sized (2
            # strips, W=64) so the final collective + its o_proj sliver are
            # all that trail the last attention strip. Contiguous token runs
            # keep the SBUF->DRAM staging DMAs at >=128B packets.
            # (b, first strip, #strips, out_e row base)
            COLLS = [(0, 0, 4, 0), (0, 4, 4, 128), (1, 0, 4, 256),
                     (1, 4, 4, 384)]
            a2a_in = [dram.tile([NC, 256, 32 * s], BF, name=f"a2a_in{i}")
                      for i, (_, _, s, _) in enumerate(COLLS)]
            a2a_out = [dram.tile([NC, 256, 32 * s], BF, name=f"a2a_out{i}")
                       for i, (_, _, s, _) in enumerate(COLLS)]

            def coll_of(b, qc):
                return 2 * b + qc // 4

            # o_proj weights: tiles reserved early (pool-nesting order), DMA
            # emitted mid-Phase A so it doesn't crowd the startup stream.
            es_wo = ExitStack()
            wop = es_wo.enter_context(tc.tile_pool(name="wop", bufs=1))
            wo_res = [wop.tile([128, NH, 1024], BF, name=f"wo{half}")
                      for half in range(2)]

            es = ExitStack()
            big = es.enter_context(tc.tile_pool(name="big", bufs=1))
            # Persistent activations (my heads, all tokens).
            # q8/k8 hold the d dim split in two 64-row k-tiles for DoubleRow
            # matmuls; q8 packs both heads per 256-token strip so a strip's
            # moving operand is one contiguous [64, 2, 512] slice. 512 slack
            # columns let the strided RoPE write slice [base, base+1024).
            q8_sb = big.tile([64, 2, B * NQC * 512 + 512], F8)
            k8_sb = big.tile([64, 2, TT], F8)              # [d%64, d//64, tok]
            v_sb = big.tile([128, TT // 128, 128], BF)     # V natural, [tok-chunk, d]

            # attention-side pools stay open through the o_proj-p0 overlap
            es_att = ExitStack()
            psS = es_att.enter_context(tc.tile_pool(name="psS", bufs=3, space="PSUM"))
            psPV = es_att.enter_context(tc.tile_pool(name="psPV", bufs=1, space="PSUM"))
            psDen = es_att.enter_context(tc.tile_pool(name="psDen", bufs=1, space="PSUM"))
            pt_pool = es_att.enter_context(tc.tile_pool(name="pt", bufs=5))
            attev = es_att.enter_context(tc.tile_pool(name="attev", bufs=2))
            mt_pool = (es_att.enter_context(tc.tile_pool(name="mt", bufs=3))
                       if mode == "generic" else None)

            def strip_chunks(b, qc):
                """Generator: one yield per pipeline stage. PV/den for chunk
                ci are emitted at stage ci+1 so they sit behind the next QK
                in the PE queue, past the exp latency."""
                cmax = 2 * qc + 2 if mode == "causal" else NKC
                mv = q8_sb[:, :, (b * NQC + qc) * 512:(b * NQC + qc) * 512 + 512]
                pv = psPV.tile([128, 512], F32, name="pv", tag="pv")
                den = psDen.tile([128, 512], F32, name="den", tag="den")
                pend = []          # un-reduced pt tiles (4-way DVE reduction)
                nden = (cmax + 3) // 4
                dci = 0

                # causal mode: the odd diagonal chunk (ci == 2qc+1, always the
                # last) fully masks query offsets [0,128) of both heads, so
                # its QK/exp/PV/den run at HALF width, accumulating into the
                # query-half-2 slices of pv/den.
                def half_q(ap3):
                    return ap3.rearrange("p (h q) -> p h q", q=256)[:, :, 128:256]

                def pv_den(ci, pt):
                    nonlocal dci
                    if mode == "causal" and ci == cmax - 1:
                        nc.tensor.matmul(half_q(pv[:]), v_sb[:, NKC * b + ci, :],
                                         pt[:], start=False, stop=True)
                        while len(pend) > 1:
                            ps2 = pt_pool.tile([128, 512], BF, name="ps2", tag="ps2")
                            nc.vector.tensor_add(ps2[:], pend[-2][:], pend[-1][:])
                            pend[-2:] = [ps2]
                        if pend:
                            nc.tensor.matmul(den[:], ones_t[:], pend[0][:],
                                             start=(dci == 0), stop=False)
                            pend.clear()
                            dci += 1
                        nc.tensor.matmul(half_q(den[:]), ones_t[:], pt[:],
                                         start=False, stop=True)
                        return
                    nc.tensor.matmul(pv[:], v_sb[:, NKC * b + ci, :], pt[:],
                                     start=(ci == 0), stop=(ci == cmax - 1))
                    if len(pend) == 2:
                        ps2 = pt_pool.tile([128, 512], BF, name="ps2", tag="ps2")
                        nc.vector.tensor_add(ps2[:], pend[0][:], pend[1][:])
                        pend[:] = [ps2]
                    pend.append(pt)
                    if ci % 4 == 3 or ci == cmax - 1:
                        if len(pend) == 2:
                            ps2 = pt_pool.tile([128, 512], BF, name="ps2", tag="ps2")
                            nc.vector.tensor_add(ps2[:], pend[0][:], pend[1][:])
                            pend[:] = [ps2]
                        nc.tensor.matmul(den[:], ones_t[:], pend[0][:],
                                         start=(dci == 0),
                                         stop=(dci == nden - 1 and mode != "causal"))
                        pend.clear()
                        dci += 1

                prev = None
                for ci in range(cmax):
                    if mode == "causal" and ci == 2 * qc + 1:
                        st = psS.tile([128, 256], F32, name="st", tag="st")
                        base = (b * NQC + qc) * 512
                        mvh = q8_sb[:, :, base:base + 512] \
                            .rearrange("p k (h q) -> p k h q", q=256)[:, :, :, 128:256]
                        nc.tensor.matmul(
                            st[:], k8_sb[:, :, b * T + 128 * ci: b * T + 128 * ci + 128],
                            mvh, start=True, stop=True, perf_mode=DR)
                        st3 = st[:].rearrange("p (h q) -> p h q", q=128)
                        nc.vector.tensor_add(st3, st3, pat_t[:, 1, :, 128:256])
                        pt = pt_pool.tile([128, 256], BF, name="pt", tag="pt")
                    else:
                        st = psS.tile([128, 512], F32, name="st", tag="st")
                        nc.tensor.matmul(st[:], k8_sb[:, :, b * T + 128 * ci: b * T + 128 * ci + 128],
                                         mv, start=True, stop=True, perf_mode=DR)
                        if mode == "causal" and ci == 2 * qc:
                            nc.vector.tensor_add(
                                st[:], st[:],
                                pat_t[:, 0, :, :].rearrange("p h t -> p (h t)"))
                        elif mode == "generic":
                            mt = mt_pool.tile([128, 256], F32, name="mt", tag="mt")
                            nc.sync.dma_start(
                                mt[:], maskT_e[128 * ci:128 * ci + 128,
                                               256 * qc:256 * qc + 256])
                            nc.vector.tensor_add(st[:, 0:256], st[:, 0:256], mt[:])
                            nc.vector.tensor_add(st[:, 256:512], st[:, 256:512], mt[:])
                        pt = pt_pool.tile([128, 512], BF, name="pt", tag="pt")
                    nc.scalar.activation(pt[:], st[:],
                                         mybir.ActivationFunctionType.Exp,
                                         scale=EXP_SCALE)
                    if prev is not None:
                        pv_den(*prev)
                    prev = (ci, pt)
                    yield
                pv_den(*prev)
                # den rows are all identical (ones stationary) == softmax denom
                den_rb = attev.tile([128, 512], F32, name="den_rb", tag="den_rb")
                nc.vector.reciprocal_approx_fast(den_rb[:], den[:])
                ao = attev.tile([128, 512], BF, name="ao", tag="ao")
                nc.vector.tensor_mul(ao[:], pv[:], den_rb[:])
                # strip (b,qc) covers tokens [qc*256, +256) of batch b; inside
                # its collective ci core j owns contiguous block [j*W, +W).
                # Staged on the scalar DGE ring so the collective triggers
                # never wait behind the big sync-ring loads or the
                # collective-gated att_g reads.
                ci = coll_of(b, qc)
                _, q0, S, _ = COLLS[ci]
                W = 32 * S
                ns = 256 // W  # slots this strip covers
                for h in range(2):  # ONE 3-dim DMA per head: each dma_start
                    # costs ~0.5us of scalar-engine issue time, so coalesce
                    nc.scalar.dma_start(
                        a2a_in[ci][(qc - q0) * ns:(qc - q0 + 1) * ns,
                                   128 * h:128 * h + 128, :]
                        .rearrange("s p q -> p s q"),
                        ao[:, 256 * h:256 * h + 256]
                        .rearrange("p (s q) -> p s q", q=W))
                done_strips[ci] += 1

            pending = deque()  # (batch, generator) of in-flight strips
            done_strips = {ci: 0 for ci in range(len(COLLS))}
            a2a_done = {ci: False for ci in range(len(COLLS))}

            def fire_ready_a2a():
                for ci, n in done_strips.items():
                    if n == COLLS[ci][2] and not a2a_done[ci]:
                        a2a_done[ci] = True
                        nc.gpsimd.collective_compute(
                            "AllToAll", mybir.AluOpType.bypass,
                            replica_groups=[list(range(NC))],
                            ins=[a2a_in[ci][:].opt()],
                            outs=[a2a_out[ci][:].opt()])

            budget = [10 ** 9]  # per-group stage cap; levels DVE/scalar load

            def drain(n):
                done = 0
                while pending and done < n and budget[0] > 0:
                    try:
                        next(pending[0][1])
                        done += 1
                        budget[0] -= 1
                    except StopIteration:
                        pending.popleft()
                        # fire each collective at the earliest emission point
                        # -- minimizes how much other queue traffic its
                        # trigger semaphores transitively wait on
                        fire_ready_a2a()

            def drain_all():
                budget[0] = 10 ** 9
                while pending:
                    drain(1)

            # -------- Phase A+B interleaved: projection feeds attention ------
            es_proj = ExitStack()
            wrope = es_proj.enter_context(tc.tile_pool(name="wrope", bufs=1))
            ht8_pool = es_proj.enter_context(tc.tile_pool(name="ht8", bufs=2))
            htB_pool = es_proj.enter_context(tc.tile_pool(name="htB", bufs=2))
            psA = es_proj.enter_context(tc.tile_pool(name="psA", bufs=2, space="PSUM"))
            psTr = es_proj.enter_context(tc.tile_pool(name="psTr", bufs=1, space="PSUM"))
            rtmp = es_proj.enter_context(tc.tile_pool(name="ropetmp", bufs=2))
            vtmp = es_proj.enter_context(tc.tile_pool(name="vtmp", bufs=2))

            ropeC_t = wrope.tile([128, T], BF)
            ropeS_t = wrope.tile([128, T], BF)
            # one tile per stream: a multi-DMA load into a single tile
            # collapses to a whole-tile dependency, which would gate the
            # first matmul on ALL of w8 instead of just stream 0
            w8s = [wrope.tile([128, NC2, 2, 128], F8, name=f"w8s{s}")
                   for s in range(3)]
            wv_t = wrope.tile([128, NHC, 128], BF)

            h8_next = None
            for g in range(NG):
                t0 = g * TN
                b_g = g // (T // TN)
                # cap stage drains so a heavy strip pair (up to 30 chunks)
                # spills into the next, lighter group instead of saturating
                # this group's DVE/scalar
                budget[0] = 24
                if g == 0:
                    # first-needed first on the sync ring: stream-0 weights,
                    # group-0 fp8 activations chunk-by-chunk (separate tiles
                    # so the first matmul starts on partial data), remaining
                    # weights. One-time loads (rope tables, v weights,
                    # consts) ride the idle scalar ring so they don't push
                    # group-1's loads back.
                    nc.sync.dma_start(w8s[0][:], w8_e[:, 0])
                    nc.scalar.dma_start(ropeC_t[:], ropeC_e[:])
                    nc.scalar.dma_start(ropeS_t[:], ropeS_e[:])
                    # 4 double-chunks (fewer dma_start issues, each ~0.5us of
                    # engine time); streams 1/2's weights right after the
                    # first chunk so no stream ever waits on weights
                    h8_parts = [ht8_pool.tile([128, 2, 2, TN], F8,
                                              name=f"h8p{p}", bufs=1)
                                for p in range(4)]
                    nc.sync.dma_start(h8_parts[0][:], hT8_e[0][:, 0:2])
                    nc.sync.dma_start(w8s[1][:], w8_e[:, 1])
                    nc.sync.dma_start(w8s[2][:], w8_e[:, 2])
                    for p in range(1, 4):
                        nc.sync.dma_start(h8_parts[p][:],
                                          hT8_e[0][:, 2 * p:2 * p + 2])
                    h8_at = lambda c2: h8_parts[c2 // 2][:, c2 % 2]
                    nc.scalar.dma_start(wv_t[:], wv_e[:])
                    nc.scalar.dma_start(ident_t[:], ident_e[:])
                    nc.scalar.dma_start(ones_t[:], ones_e[:])
                    if mode == "causal":
                        nc.scalar.dma_start(
                            pat_t[:], pat_e[:].rearrange("s p h t -> p s h t"))
                else:
                    h8_g = h8_next
                    h8_at = lambda c2: h8_g[:, c2]
                # prefetch the NEXT group's fp8 activations ahead of this
                # group's v-stream load: the q/k streams of g+1 never wait.
                # Exception g0: its own ht gates the v stream ~8us earlier
                # than g1 needs h8, so ht goes first there.
                def prefetch_h8():
                    nonlocal h8_next
                    h8_next = ht8_pool.tile([128, NC2, 2, TN], F8, name="h8",
                                            tag="h8")
                    nc.sync.dma_start(h8_next[:], hT8_e[g + 1])
                if 0 < g < NG - 1:
                    prefetch_h8()
                ht_g = htB_pool.tile([128, NHC, TN], BF, name="ht", tag="ht")
                nc.sync.dma_start(ht_g[:], hTB_e[g])
                if g == 0:
                    prefetch_h8()
                ht_at = lambda hc: ht_g[:, hc, :]
                if g in (3, 5):
                    half = (g - 3) // 2
                    nc.sync.dma_start(
                        wo_res[half][:],
                        woT_e[:, half * 1024:(half + 1) * 1024]
                        .rearrange("(h p) n -> p h n", p=128))
                ctab = g % (T // TN) * TN  # rope table column offset

                def rope_out(s, ps):
                    # RoPE: out = ps*C + rot(ps)*S  (S carries the sign),
                    # written as fp8 d-halves for the DoubleRow QK matmul.
                    # The chain runs in bf16 (2x DVE rate) behind one scalar
                    # copy; q/k noise only lands on attention scores.
                    pb = rtmp.tile([128, TN], BF, name="pb", tag="pb")
                    nc.scalar.copy(pb[:], ps[:])
                    csl = ropeC_t[:, ctab:ctab + TN]
                    ssl = ropeS_t[:, ctab:ctab + TN]
                    t1 = rtmp.tile([128, TN], BF, name="t1", tag="t1")
                    t2 = rtmp.tile([128, TN], BF, name="t2", tag="t2")
                    # ropeS halves are swapped host-side so each mul reads
                    # both SBUF inputs from the same base partition
                    nc.vector.tensor_mul(t1[:], pb[:], csl)
                    nc.vector.tensor_mul(t2[0:64, :], pb[64:128, :], ssl[64:128, :])
                    nc.vector.tensor_mul(t2[64:128, :], pb[0:64, :], ssl[0:64, :])
                    if s < 2:
                        # strip-packed layout: head s of strip qc occupies
                        # tokens [qc*512 + s*256, +256) -- this group's two
                        # strips land at base+{0,512} (stride-512 write)
                        base = (b_g * NQC + 2 * (g % 4)) * 512 + s * 256
                        for k in range(2):
                            dst = q8_sb[:, k, base:base + 1024] \
                                .rearrange("p (q t) -> p q t", t=512)[:, :, 0:256]
                            nc.vector.tensor_add(
                                dst,
                                t1[64 * k:64 * k + 64, :].rearrange("p (q t) -> p q t", t=256),
                                t2[64 * k:64 * k + 64, :].rearrange("p (q t) -> p q t", t=256))
                    else:
                        for k in range(2):
                            nc.vector.tensor_add(
                                k8_sb[:, k, t0:t0 + TN],
                                t1[64 * k:64 * k + 64, :],
                                t2[64 * k:64 * k + 64, :])

                if g == 0:
                    # cold start: interleave the three streams per chunk so
                    # the PE consumes activation chunks at DMA delivery rate
                    # (a stream-at-a-time order races ahead and stalls). The
                    # third accumulator borrows the psPV bank, idle until the
                    # first strip drains in group 1.
                    pss = [psA.tile([128, TN], F32, name="psA", tag="psA"),
                           psA.tile([128, TN], F32, name="psA", tag="psA"),
                           psPV.tile([128, TN], F32, name="pv", tag="pv")]
                    for c2 in range(NC2):
                        for s in range(3):
                            nc.tensor.matmul(pss[s][:], w8s[s][:, c2],
                                             h8_at(c2), start=(c2 == 0),
                                             stop=(c2 == NC2 - 1),
                                             perf_mode=DR)
                    for s in range(3):
                        rope_out(s, pss[s])
                else:
                    for s in range(3):  # q1, q2, k in fp8 DoubleRow
                        ps = psA.tile([128, TN], F32, name="psA", tag="psA")
                        for c2 in range(NC2):
                            nc.tensor.matmul(ps[:], w8s[s][:, c2], h8_at(c2),
                                             start=(c2 == 0),
                                             stop=(c2 == NC2 - 1),
                                             perf_mode=DR)
                            drain(1)
                        rope_out(s, ps)
                        drain(1)
                # v stream in bf16 (its noise lands on the output directly).
                # g0 borrows the psDen bank (first used by strip (0,0)'s den
                # flush mid-g1) so both psA slots recycle to g1's streams off
                # the fast pb copies instead of waiting for g0's vt copy.
                if g == 0:
                    ps = psDen.tile([128, TN], F32, name="den", tag="den")
                else:
                    ps = psA.tile([128, TN], F32, name="psA", tag="psA")
                for hc in range(NHC):
                    nc.tensor.matmul(ps[:], wv_t[:, hc], ht_at(hc),
                                     start=(hc == 0), stop=(hc == NHC - 1))
                    drain(1)
                # V^T -> transpose to V natural via PE
                vt = vtmp.tile([128, TN], BF, name="vt", tag="vt")
                nc.scalar.copy(vt[:], ps[:])
                for j in range(TN // 128):
                    trp = psTr.tile([128, 128], BF, name="trp", tag="trp")
                    nc.tensor.transpose(trp[:], vt[:, j * 128:(j + 1) * 128], ident_t[:])
                    nc.vector.tensor_copy(v_sb[:, g * (TN // 128) + j, :], trp[:])
                    drain(1)
                # enqueue strips unlocked by this group (drained next group)
                if mode == "causal":
                    strips = [(b_g, 2 * (g % 4)), (b_g, 2 * (g % 4) + 1)]
                else:
                    strips = ([(g // 4, qc) for qc in range(NQC)]
                              if g in (3, 7) else [])
                for b, qc in strips:
                    pending.append((b, strip_chunks(b, qc)))
                fire_ready_a2a()

            es_proj.close()  # free projection SBUF + the psA/psTr PSUM banks

            # ---------------- Phase C: o_proj --------------------------------
            # One att_g tile per collective so each o_proj slice depends ONLY
            # on its own collective; passes are emitted in arrival order.
            with tc.tile_pool(name="attg", bufs=1) as attg_pool, \
                 tc.tile_pool(name="fin", bufs=3, space="PSUM") as fin_pool, \
                 tc.tile_pool(name="fo", bufs=3) as fo_pool:

                att_g = [attg_pool.tile([128, NH, 32 * s], BF, name=f"attg{i}")
                         for i, (_, _, s, _) in enumerate(COLLS)]

                def attg_dma(ci):
                    # alternate hw queues so the 8 transfers drain in
                    # parallel right after the collective lands
                    for j in range(NC):
                        eng = nc.sync if j % 2 == 0 else nc.scalar
                        eng.dma_start(
                            att_g[ci][:, 2 * j:2 * j + 2, :],
                            a2a_out[ci][j].rearrange("(h p) t -> p h t", p=128))

                def oproj_sub(ci, half, n2, split_out=False):
                    W = 32 * COLLS[ci][2]
                    row = COLLS[ci][3]
                    col = half * 1024 + n2 * 512
                    fin = fin_pool.tile([128, 512], F32, name="fin", tag="fin")
                    for h in range(NH):
                        nc.tensor.matmul(
                            fin[0:W, :],
                            att_g[ci][:, h, :],
                            wo_res[half][:, h, n2 * 512:(n2 + 1) * 512],
                            start=(h == 0), stop=(h == NH - 1))
                    fo = fo_pool.tile([128, 512], BF, name="fo", tag="fo")
                    if split_out:
                        # final sub-block: split the flush across both DMA
                        # rings; copies on the (idle-by-now) DVE
                        nc.vector.tensor_copy(fo[0:W, 0:256], fin[0:W, 0:256])
                        nc.vector.tensor_copy(fo[0:W, 256:512], fin[0:W, 256:512])
                        nc.sync.dma_start(out_e[row:row + W, col:col + 256],
                                          fo[0:W, 0:256])
                        nc.scalar.dma_start(out_e[row:row + W, col + 256:col + 512],
                                            fo[0:W, 256:512])
                        return
                    # the last slice runs after the strips are done: the DVE
                    # is idle there, while the scalar queue still drains attg
                    # issues -- keep its copies off the scalar engine
                    if (half + n2) % 2 == 0 or ci == len(COLLS) - 1:
                        nc.vector.tensor_copy(fo[0:W, :], fin[0:W, :])
                    else:
                        nc.scalar.copy(fo[0:W, :], fin[0:W, :])
                    nc.sync.dma_start(
                        out_e[row:row + W, col:col + 512], fo[0:W, :])

                # drain the last strips WITHOUT consuming o_proj work; each
                # o_proj slice then waits only on its own collective. One
                # sub-block of each of the first three slices is DEFERRED to
                # bridge the final collective's latency with PE work that is
                # already unlocked.
                drain_all()
                defer = [(0, 1, 1), (1, 1, 1), (2, 1, 1)]
                for ci in range(len(COLLS) - 1):
                    attg_dma(ci)
                    for half in range(2):
                        for n2 in range(2):
                            if (ci, half, n2) not in defer:
                                oproj_sub(ci, half, n2)
                for ci, half, n2 in defer:
                    oproj_sub(ci, half, n2)
                ci = len(COLLS) - 1
                attg_dma(ci)
                for half in range(2):
                    for n2 in range(2):
                        oproj_sub(ci, half, n2,
                                  split_out=(half == 1 and n2 == 1))
            es_att.close()
            es.close()
            es_wo.close()

    nc.compile()
    return nc


def _host_prep(hidden_states, freqs_cos, freqs_sin, mask, w_qkv, w_o, kv_write_indices):
    idx = np.asarray(kv_write_indices).astype(np.int64)
    if not np.array_equal(idx, np.arange(T, dtype=np.int64)):
        raise NotImplementedError("kernel specialized for kv_write_indices == arange(T)")

    hs = np.asarray(hidden_states, dtype=np.float32).reshape(TT, HID)
    hsT = hs.T  # [HID, TT]
    # bf16 copy (v stream): [HID, TT] -> [NG, 128, NHC, TN] (one DMA per group)
    hTB = np.ascontiguousarray(
        hsT.reshape(NHC, 128, NG, TN).transpose(2, 1, 0, 3)).astype(NPBF)
    # fp8 copy (q/k streams): chunk pairs for DoubleRow, one DMA per group
    # [NG, 128, NC2, 2, TN]; element (g,p,c2,k,t) = hsT[c2*256+k*128+p, g*TN+t]
    hT8 = np.ascontiguousarray(
        (hsT * S_H).reshape(NC2, 2, 128, NG, TN).transpose(3, 2, 0, 1, 4)).astype(NPF8)

    m2 = np.asarray(mask, dtype=np.float32).reshape(T, T)
    tril = np.tril(np.ones((T, T), dtype=bool))
    if not m2.any():
        mode = "none"
    elif (m2[tril] == 0).all() and (m2[~tril] <= -1e30).all():
        mode = "causal"
    else:
        mode = "generic"

    wq = np.asarray(w_qkv, dtype=np.float32)
    woT = np.ascontiguousarray(np.asarray(w_o, dtype=np.float32).T).astype(NPBF)

    def tile_w8(wrows):
        # [128 out, HID] -> [NC2, 2, 128 hid, 128 out] DoubleRow stationary
        return np.ascontiguousarray(wrows.T).reshape(NC2, 2, 128, 128)

    w8s, wvs = [], []
    for c in range(NC):
        q1 = wq[(2 * c) * HD:(2 * c + 1) * HD] * (SCALE * S_WQ)
        q2 = wq[(2 * c + 1) * HD:(2 * c + 2) * HD] * (SCALE * S_WQ)
        k = wq[NH * HD + c * HD: NH * HD + (c + 1) * HD] * S_WK
        v = wq[(NH + NKV) * HD + c * HD: (NH + NKV) * HD + (c + 1) * HD]
        # [3, NC2, 2, 128 hid, 128 out] -> [128 hid, 3, NC2, 2, 128 out]
        w8s.append(np.ascontiguousarray(
            np.stack([tile_w8(q1), tile_w8(q2), tile_w8(k)])
            .transpose(3, 0, 1, 2, 4)).astype(NPF8))
        # v: [NHC, 128 hid, 128 out] -> [128 hid, NHC, 128 out]
        wvs.append(np.ascontiguousarray(
            v.T.reshape(NHC, 128, 128).transpose(1, 0, 2)).astype(NPBF))

    cosT = np.asarray(freqs_cos, dtype=np.float32).T  # [64, T]
    sinT = np.asarray(freqs_sin, dtype=np.float32).T
    ropeC = np.ascontiguousarray(np.concatenate([cosT, cosT], axis=0)).astype(NPBF)
    # halves swapped: rows 0:64 feed t2[64:128] (+sin), rows 64:128 feed
    # t2[0:64] (-sin) -- keeps both DVE inputs on one base partition
    ropeS = np.ascontiguousarray(np.concatenate([sinT, -sinT], axis=0)).astype(NPBF)

    consts = {
        "ropeC": ropeC,
        "ropeS": ropeS,
        "ones": np.ones((128, 128), NPBF),
        "ident": np.eye(128, dtype=np.float32).astype(NPBF),
    }
    if mode == "causal":
        kr = np.arange(256)[:, None]
        qr = np.arange(256)[None, :]
        pat = np.where(kr <= qr, np.float32(0.0), np.float32(NEG)).astype(np.float32)
        pat = pat.reshape(2, 128, 1, 256).repeat(2, axis=2)  # dup over heads
        consts["pat"] = np.ascontiguousarray(pat).astype(NPBF)
    elif mode == "generic":
        # scores sit in PSUM pre-multiplied by 1/EXP_SCALE; match the mask
        consts["maskT"] = np.ascontiguousarray(
            np.clip(m2.T * (1.0 / EXP_SCALE), -3.0e38, 3.0e38))

    in_maps = []
    for c in range(NC):
        m = {"hT8": hT8, "hTB": hTB, "w8": w8s[c], "wv": wvs[c], "woT": woT}
        m.update(consts)
        in_maps.append(m)
    return mode, in_maps


def kernel(hidden_states, freqs_cos, freqs_sin, k_cache, v_cache, mask, w_qkv,
           w_o, kv_write_indices):
    # k_cache/v_cache are fully overwritten (kv_write_indices == arange covers
    # every slot), so their incoming contents are irrelevant.
    global last_results
    mode, in_maps = _host_prep(hidden_states, freqs_cos, freqs_sin, mask,
                               w_qkv, w_o, kv_write_indices)
    if mode not in _CACHE:
        _CACHE[mode] = _build(mode)
    nc = _CACHE[mode]

    trace = bool(os.environ.get("BASS_KERNEL_TRACE"))
    res = run_bass_kernel_spmd(nc, in_maps, core_ids=list(range(NC)), trace=trace)
    last_results = res

    # token-split resharding: out rows per collective, core c owning the
    # contiguous block [base_tok + c*W, +W) of that collective's batch
    # (b, base_tok within batch, W, out row base)
    colls = [(0, 0, 128, 0), (0, 1024, 128, 128), (1, 0, 128, 256),
             (1, 1024, 128, 384)]
    final = np.empty((B, T, HID), dtype=np.float32)
    for c in range(NC):
        o = np.asarray(res.results[c]["out"], dtype=np.float32)
        for b, bt, w, row in colls:
            final[b, bt + c * w: bt + (c + 1) * w] = o[row:row + w]
    return final

